# revision 47
# baseline (speedup 1.0000x reference)
"""Trainium2 Bass kernel for fused Llama attention (nn_LlamaAttentionFused).

Reference computation (B=2, S=1024, H=4096, 32 Q heads, 8 KV heads, D=128):
    xq = x @ wq; xk = x @ wk; xv = x @ wv
    rope(xq, xk); causal GQA flash attention; out = attn @ wo

Sharding: 8-way tensor parallel over heads. Core c owns Q heads 4c..4c+3 and
KV head c (GQA groups stay together). Each core computes a full-shape partial
output (its heads' contribution through wo); the host sums the 8 partials.

v2 design (bf16 end-to-end, fp32 PSUM accumulation):
  - All weights resident in SBUF (10MB bf16); x streamed once.
  - Scores computed directly transposed: sT[k,q] = kT_blk^T-stationary @ qT
    so no probs transposes are needed for the PV matmul.
  - Softmax without row-max subtraction (|logits| <~ 12, exp fits fp32
    comfortably); exp per 512-col chunk straight out of PSUM into bf16.
  - Denominator for free: V is augmented with a ones column, so the PV
    matmul's last output column accumulates sum_k probs[k,q]. The renorm
    1/den is then a per-partition tensor_scalar at PV evacuation.
  - RoPE as 3 full-width DVE ops: host bakes -sin into the top half of the
    sin table so both halves use the same add.
  - DMAs split into ~0.25-0.5MB chunks (each DMA queue sustains ~21GB/s).

Device-side layouts (per core):
    xT   [4096, 2048]  x transposed on host (tokens = 2 batches x 1024)
    wq   [4096, 512]   natural (stationary [K=H, M=dims])
    wkv  [4096, 256]   wk|wv column-concat
    wo   [512, 4096]   natural (moving operand)
    cosf [128, 1024]   freqs_cos.T stacked twice on the partition axis
    sinf [128, 1024]   [-freqs_sin.T ; +freqs_sin.T]
    out  [2048, 4096]  partial output (bf16; host sums in fp32)
"""

import numpy as np
import ml_dtypes

import concourse.bass as bass
import concourse.mybir as mybir
import concourse.tile as tile
from concourse import bacc
from concourse.bass_utils import run_bass_kernel_spmd
from concourse.masks import make_identity

F32 = mybir.dt.float32
BF16 = mybir.dt.bfloat16

B = 2
S = 1024          # tokens per batch
H = 4096          # model dim
D = 128           # head dim
HQ = 4            # q heads per core
NT = B * S        # total tokens
SCALE = 1.0 / float(np.sqrt(D))
NEG = -1.0e30     # additive causal mask value (pre-scale)

QB = S // 128     # 8 q-blocks of 128 per batch
KC = S // 128     # 8 k-blocks of 128 per batch
HC = H // 128     # 32 contraction chunks for the projections


def build_program():
    nc = bacc.Bacc("TRN2", target_bir_lowering=False, debug=False, num_devices=8)

    xT = nc.dram_tensor("xT", [H, NT], BF16, kind="ExternalInput").ap()
    wq = nc.dram_tensor("wq", [H, HQ * D], BF16, kind="ExternalInput").ap()
    wkv = nc.dram_tensor("wkv", [H, 2 * D], BF16, kind="ExternalInput").ap()
    wo = nc.dram_tensor("wo", [HQ * D, H], BF16, kind="ExternalInput").ap()
    cosf = nc.dram_tensor("cosf", [128, S], BF16, kind="ExternalInput").ap()
    sinf = nc.dram_tensor("sinf", [128, S], BF16, kind="ExternalInput").ap()
    out = nc.dram_tensor("out", [NT, H], BF16, kind="ExternalOutput").ap()

    xT_r = xT.rearrange("(n p) f -> p n f", p=128)     # [128, 32, 2048]
    wq_r = wq.rearrange("(n p) f -> p n f", p=128)     # [128, 32, 512]
    wkv_r = wkv.rearrange("(n p) f -> p n f", p=128)   # [128, 32, 256]
    wo_r = wo.rearrange("(n p) f -> p n f", p=128)     # [128, 4, 4096]

    with tile.TileContext(nc) as tc:
        with (
            tc.tile_pool(name="const", bufs=1) as const,
            tc.tile_pool(name="weights", bufs=1) as weights,
            tc.tile_pool(name="stream", bufs=6) as stream,
            tc.tile_pool(name="acts", bufs=1) as acts,
            tc.tile_pool(name="ropes", bufs=3) as ropes,
            tc.tile_pool(name="probs", bufs=3) as probs,
            tc.tile_pool(name="an", bufs=6) as anpool,
            tc.tile_pool(name="dmat", bufs=6) as dmat,
            tc.tile_pool(name="ev", bufs=2) as evpool,
            tc.tile_pool(name="stats", bufs=16) as stats,
            tc.tile_pool(name="ps", bufs=7, space="PSUM") as psmain,
        ):
            # ---- constants -------------------------------------------------
            ident = const.tile([128, 128], BF16)
            make_identity(nc, ident)

            # maskT[p, f] = 0 where p <= f (k <= q valid), NEG where k > q
            maskT = const.tile([128, 128], F32)
            nc.gpsimd.memset(maskT, 0.0)
            nc.gpsimd.affine_select(
                out=maskT,
                in_=maskT,
                compare_op=mybir.AluOpType.is_ge,
                fill=NEG,
                base=0,
                pattern=[[1, 128]],
                channel_multiplier=-1,
            )

            # ---- resident weights: DMAs are emitted just-in-time inside
            # the first projection pass (see below) so the queue order
            # matches consumption order and the PE starts ~15us in ---------
            wq_s = weights.tile([128, HC, HQ * D], BF16)
            wkv_s = weights.tile([128, HC, 2 * D], BF16)
            cosf_s = const.tile([128, S], BF16)
            sinf_s = const.tile([128, S], BF16)
            wo_s = weights.tile([128, HQ, H], BF16)

            def rope(dst, tsl):
                """In-place RoPE on dst[:, tsl] ([128, 512] bf16 slice).
                Runs on gpsimd (SBUF-only) to keep the vector queue free
                for the attention softmax chain."""
                scr = ropes.tile([128, 512], BF16, tag="scr")
                nc.sync.dma_start(out=scr[0:64, :], in_=dst[64:128, tsl])
                nc.sync.dma_start(out=scr[64:128, :], in_=dst[0:64, tsl])
                nc.gpsimd.tensor_mul(dst[:, tsl], dst[:, tsl], cosf_s[:, tsl])
                nc.gpsimd.tensor_mul(scr, scr, sinf_s[:, tsl])
                nc.gpsimd.tensor_add(dst[:, tsl], dst[:, tsl], scr)

            # Both batches' activations stay resident; phases are merged
            # as proj(b0), proj(b1), attn(b0), attn(b1), out(b0), out(b1)
            # so the second attention starts on drained engine queues.
            qTs, kTs, vTs, vnats, attnTs = [], [], [], [], []
            for b in range(B):
                qTs.append(acts.tile([128, HQ, S], BF16, tag=f"qT{b}",
                                     name=f"qT{b}"))
                kTs.append(acts.tile([128, S], BF16, tag=f"kT{b}",
                                     name=f"kT{b}"))
                vTs.append(acts.tile([128, S], BF16, tag=f"vT{b}",
                                     name=f"vT{b}"))
                vnats.append(acts.tile([128, KC, D + 1], BF16,
                                       tag=f"vnat{b}", name=f"vnat{b}"))
                attnTs.append(acts.tile([128, HQ, S], BF16, tag=f"attnT{b}",
                                        name=f"attnT{b}"))

            # ---- projections: qT/kT/vT = w.T @ x --------------------------
            for b in range(B):
                tok0 = b * S
                qT, kT, vT, vnat = qTs[b], kTs[b], vTs[b], vnats[b]
                nc.gpsimd.memset(vnat, 1.0)  # ones column for the denom

                for t in range(2):  # two 512-token chunks per batch
                    tsl = slice(t * 512, (t + 1) * 512)
                    psq = [psmain.tile([128, 512], F32, tag="ps",
                                       name=f"psq{_d}") for _d in range(HQ)]
                    psk = psmain.tile([128, 512], F32, tag="ps")
                    psv = psmain.tile([128, 512], F32, tag="ps")
                    for hcp in range(HC // 2):  # two 128-chunks per DMA
                        if b == 0 and t == 0:
                            # weights just-in-time, in consumption order
                            nc.sync.dma_start(
                                out=wq_s[:, 2 * hcp:2 * hcp + 2, :],
                                in_=wq_r[:, 2 * hcp:2 * hcp + 2, :])
                            nc.sync.dma_start(
                                out=wkv_s[:, 2 * hcp:2 * hcp + 2, :],
                                in_=wkv_r[:, 2 * hcp:2 * hcp + 2, :])
                        xp = stream.tile([128, 2, 512], BF16, tag="xp")
                        nc.sync.dma_start(
                            out=xp,
                            in_=xT_r[:, 2 * hcp:2 * hcp + 2,
                                     tok0 + t * 512: tok0 + (t + 1) * 512],
                        )
                        for sub in range(2):
                            hc = 2 * hcp + sub
                            first, last = hc == 0, hc == HC - 1
                            for d in range(HQ):
                                nc.tensor.matmul(
                                    psq[d],
                                    wq_s[:, hc, d * 128:(d + 1) * 128],
                                    xp[:, sub, :],
                                    start=first, stop=last,
                                )
                            nc.tensor.matmul(psk, wkv_s[:, hc, 0:128],
                                             xp[:, sub, :],
                                             start=first, stop=last)
                            nc.tensor.matmul(psv, wkv_s[:, hc, 128:256],
                                             xp[:, sub, :],
                                             start=first, stop=last)
                    if b == 0 and t == 0:
                        nc.sync.dma_start(out=cosf_s, in_=cosf)
                        nc.sync.dma_start(out=sinf_s, in_=sinf)
                    # evacuate + rope; k first so attention unblocks
                    # earliest. Evacs split scalar/vector so neither queue
                    # backlogs ahead of the attention softmax chain.
                    nc.vector.tensor_copy(kT[:, tsl], psk)
                    rope(kT, tsl)
                    nc.vector.tensor_copy(vT[:, tsl], psv)
                    # v natural [tok, d | 1] via PE transpose, per t-chunk
                    tp = psmain.tile([128, 512], BF16, tag="tpb", bufs=1)
                    for i in range(4):
                        kc = t * 4 + i
                        nc.tensor.transpose(
                            tp[:, i * 128:(i + 1) * 128],
                            vT[:, kc * 128:(kc + 1) * 128],
                            ident,
                        )
                    nc.vector.tensor_copy(vnat[:, t * 4:(t + 1) * 4, 0:128],
                                          tp)
                    for d in range(HQ):
                        if d < 2:
                            nc.scalar.copy(qT[:, d, tsl], psq[d])
                        else:
                            nc.vector.tensor_copy(qT[:, d, tsl], psq[d])
                        rope(qT[:, d, :], tsl)

                if b == 0:
                    # wo lands well before the first out-projection
                    for i in range(8):
                        nc.sync.dma_start(
                            out=wo_s[:, :, i * 512:(i + 1) * 512],
                            in_=wo_r[:, :, i * 512:(i + 1) * 512])

            # ---- attention: scores transposed, PV, renorm -----------------
            def scores(b, hh, qc):
                """sT[k,q] per k-block -> exp -> probsT (unnormalized)."""
                pt = probs.tile([128, KC, 512], BF16, tag="pt")
                q0 = qc * 512
                for kc in range(qc * 4 + 4):
                    qoff = max(0, kc * 128 - q0)  # causal column start
                    st = psmain.tile([128, 512], F32, tag="ps")
                    nc.tensor.matmul(
                        st[:, qoff:512],
                        kTs[b][:, kc * 128:(kc + 1) * 128],
                        qTs[b][:, hh, q0 + qoff:q0 + 512],
                        start=True, stop=True,
                    )
                    if kc * 128 >= q0:  # diagonal block: causal mask
                        nc.vector.tensor_add(
                            st[:, qoff:qoff + 128],
                            st[:, qoff:qoff + 128],
                            maskT,
                        )
                    nc.scalar.activation(
                        pt[:, kc, qoff:512],
                        st[:, qoff:512],
                        mybir.ActivationFunctionType.Exp,
                        scale=SCALE,
                    )
                return pt

            def pv_tail(b, hh, qc, pt):
                """PV per q-block, renorm by the free denominator,
                transpose back into attnT layout via XBAR DMA transpose
                (zero-offset src and dst tiles only!) to spare the PE."""
                q0 = qc * 512
                for qr in range(4):
                    qb = qc * 4 + qr
                    pa = psmain.tile([128, 512], F32, tag="ps")
                    for kc in range(qb + 1):
                        nc.tensor.matmul(
                            pa[:, 0:D + 1],
                            pt[:, kc, qr * 128:(qr + 1) * 128],
                            vnats[b][:, kc, :],
                            start=(kc == 0), stop=(kc == qb),
                        )
                    rec = stats.tile([128, 1], F32, tag="st")
                    nc.vector.reciprocal(rec, pa[:, D:D + 1])
                    an = anpool.tile([128, 128], BF16, tag="an")
                    nc.vector.tensor_scalar_mul(an, pa[:, 0:D], rec)
                    sc = dmat.tile([128, 128], BF16, tag="dt")
                    nc.sync.dma_start_transpose(sc, an)
                    nc.vector.tensor_copy(
                        attnTs[b][:, hh, q0 + qr * 128:q0 + (qr + 1) * 128],
                        sc)

            prev = None
            for b in range(B):
                for qc in range(2):
                    for hh in range(HQ):
                        pt = scores(b, hh, qc)
                        if prev is not None:
                            pv_tail(*prev)
                        prev = (b, hh, qc, pt)
            pv_tail(*prev)

            # ---- output projection: out[tok, :] += attnT.T @ wo -----------
            for b in range(B):
                tok0 = b * S
                attnT = attnTs[b]
                for tb in range(QB):
                    ev = evpool.tile([128, H], BF16, tag="ev")
                    for ncol in range(8):
                        po = psmain.tile([128, 512], F32, tag="ps")
                        for d in range(HQ):
                            nc.tensor.matmul(
                                po,
                                attnT[:, d, tb * 128:(tb + 1) * 128],
                                wo_s[:, d, ncol * 512:(ncol + 1) * 512],
                                start=(d == 0), stop=(d == HQ - 1),
                            )
                        if ncol % 2 == 0:
                            nc.scalar.copy(ev[:, ncol * 512:(ncol + 1) * 512],
                                           po)
                        else:
                            nc.vector.tensor_copy(
                                ev[:, ncol * 512:(ncol + 1) * 512], po)
                    # split across DMA queues; last tile finer for the tail
                    nsp = 8 if (b == B - 1 and tb == QB - 1) else 4
                    w = H // nsp
                    for i in range(nsp):
                        nc.sync.dma_start(
                            out=out[tok0 + tb * 128: tok0 + (tb + 1) * 128,
                                    i * w:(i + 1) * w],
                            in_=ev[:, i * w:(i + 1) * w],
                        )

    nc.compile()
    return nc


_NC = None


def _get_nc():
    global _NC
    if _NC is None:
        _NC = build_program()
    return _NC


def make_in_maps(x, wq, wk, wv, wo, freqs_cos, freqs_sin):
    bf = ml_dtypes.bfloat16
    x = np.asarray(x, np.float32)
    xT = np.ascontiguousarray(x.reshape(NT, H).T.astype(bf))
    cosT = np.asarray(freqs_cos, np.float32).T
    sinT = np.asarray(freqs_sin, np.float32).T
    cosf = np.ascontiguousarray(np.concatenate([cosT, cosT], 0).astype(bf))
    sinf = np.ascontiguousarray(np.concatenate([-sinT, sinT], 0).astype(bf))
    wq = np.asarray(wq, np.float32).astype(bf)
    wk = np.asarray(wk, np.float32).astype(bf)
    wv = np.asarray(wv, np.float32).astype(bf)
    wo = np.asarray(wo, np.float32).astype(bf)
    in_maps = []
    for c in range(8):
        in_maps.append({
            "xT": xT,
            "wq": np.ascontiguousarray(wq[:, c * 512:(c + 1) * 512]),
            "wkv": np.ascontiguousarray(
                np.concatenate([wk[:, c * 128:(c + 1) * 128],
                                wv[:, c * 128:(c + 1) * 128]], axis=1)),
            "wo": np.ascontiguousarray(wo[c * 512:(c + 1) * 512, :]),
            "cosf": cosf,
            "sinf": sinf,
        })
    return in_maps


def kernel(x, wq, wk, wv, wo, freqs_cos, freqs_sin, start_pos=0, **_):
    nc = _get_nc()
    in_maps = make_in_maps(x, wq, wk, wv, wo, freqs_cos, freqs_sin)
    res = run_bass_kernel_spmd(nc, in_maps, list(range(8)))
    acc = res.results[0]["out"].astype(np.float32)
    for c in range(1, 8):
        acc = acc + res.results[c]["out"].astype(np.float32)
    return acc.reshape(B, S, H)


# revision 49
# speedup vs baseline: 1.1312x; 1.1312x over previous
"""Trainium2 Bass kernel for fused Llama attention (nn_LlamaAttentionFused).

Reference computation (B=2, S=1024, H=4096, 32 Q heads, 8 KV heads, D=128):
    xq = x @ wq; xk = x @ wk; xv = x @ wv
    rope(xq, xk); causal GQA flash attention; out = attn @ wo

Sharding: 8-way tensor parallel over heads. Core c owns Q heads 4c..4c+3 and
KV head c (GQA groups stay together). Each core computes a full-shape partial
output (its heads' contribution through wo); the host sums the 8 partials.

v2 design (bf16 end-to-end, fp32 PSUM accumulation):
  - All weights resident in SBUF (10MB bf16); x streamed once.
  - Scores computed directly transposed: sT[k,q] = kT_blk^T-stationary @ qT
    so no probs transposes are needed for the PV matmul.
  - Softmax without row-max subtraction (|logits| <~ 12, exp fits fp32
    comfortably); exp per 512-col chunk straight out of PSUM into bf16.
  - Denominator for free: V is augmented with a ones column, so the PV
    matmul's last output column accumulates sum_k probs[k,q]. The renorm
    1/den is then a per-partition tensor_scalar at PV evacuation.
  - RoPE as 3 full-width DVE ops: host bakes -sin into the top half of the
    sin table so both halves use the same add.
  - DMAs split into ~0.25-0.5MB chunks (each DMA queue sustains ~21GB/s).

Device-side layouts (per core):
    xT   [4096, 2048]  x transposed on host (tokens = 2 batches x 1024)
    wq   [4096, 512]   natural (stationary [K=H, M=dims])
    wkv  [4096, 256]   wk|wv column-concat
    wo   [512, 4096]   natural (moving operand)
    cosf [128, 1024]   freqs_cos.T stacked twice on the partition axis
    sinf [128, 1024]   [-freqs_sin.T ; +freqs_sin.T]
    out  [2048, 4096]  partial output (bf16; host sums in fp32)
"""

import numpy as np
import ml_dtypes

import concourse.bass as bass
import concourse.mybir as mybir
import concourse.tile as tile
from concourse import bacc
from concourse.bass_utils import run_bass_kernel_spmd
from concourse.masks import make_identity

F32 = mybir.dt.float32
BF16 = mybir.dt.bfloat16

B = 2
S = 1024          # tokens per batch
H = 4096          # model dim
D = 128           # head dim
HQ = 4            # q heads per core
NT = B * S        # total tokens
SCALE = 1.0 / float(np.sqrt(D))
NEG = -1.0e30     # additive causal mask value (pre-scale)

QB = S // 128     # 8 q-blocks of 128 per batch
KC = S // 128     # 8 k-blocks of 128 per batch
HC = H // 128     # 32 contraction chunks for the projections


def build_program():
    nc = bacc.Bacc("TRN2", target_bir_lowering=False, debug=False, num_devices=8)

    xT = nc.dram_tensor("xT", [H, NT], BF16, kind="ExternalInput").ap()
    wq = nc.dram_tensor("wq", [H, HQ * D], BF16, kind="ExternalInput").ap()
    wkv = nc.dram_tensor("wkv", [H, 2 * D], BF16, kind="ExternalInput").ap()
    wo = nc.dram_tensor("wo", [HQ * D, H], BF16, kind="ExternalInput").ap()
    cosf = nc.dram_tensor("cosf", [128, S], BF16, kind="ExternalInput").ap()
    sinf = nc.dram_tensor("sinf", [128, S], BF16, kind="ExternalInput").ap()
    out = nc.dram_tensor("out", [NT, H], BF16, kind="ExternalOutput").ap()

    xT_r = xT.rearrange("(n p) f -> p n f", p=128)     # [128, 32, 2048]
    wq_r = wq.rearrange("(n p) f -> p n f", p=128)     # [128, 32, 512]
    wkv_r = wkv.rearrange("(n p) f -> p n f", p=128)   # [128, 32, 256]
    wo_r = wo.rearrange("(n p) f -> p n f", p=128)     # [128, 4, 4096]

    with tile.TileContext(nc) as tc:
        with (
            tc.tile_pool(name="const", bufs=1) as const,
            tc.tile_pool(name="weights", bufs=1) as weights,
            tc.tile_pool(name="stream", bufs=8) as stream,
            tc.tile_pool(name="acts", bufs=1) as acts,
            tc.tile_pool(name="ropes", bufs=4) as ropes,
            tc.tile_pool(name="probs", bufs=4) as probs,
            tc.tile_pool(name="an", bufs=8) as anpool,
            tc.tile_pool(name="ev", bufs=3) as evpool,
            tc.tile_pool(name="stats", bufs=24) as stats,
            tc.tile_pool(name="ps", bufs=7, space="PSUM") as psmain,
        ):
            # ---- constants -------------------------------------------------
            ident = const.tile([128, 128], BF16)
            make_identity(nc, ident)

            # maskT[p, f] = 0 where p <= f (k <= q valid), NEG where k > q
            maskT = const.tile([128, 128], F32)
            nc.gpsimd.memset(maskT, 0.0)
            nc.gpsimd.affine_select(
                out=maskT,
                in_=maskT,
                compare_op=mybir.AluOpType.is_ge,
                fill=NEG,
                base=0,
                pattern=[[1, 128]],
                channel_multiplier=-1,
            )

            # ---- resident weights: DMAs are emitted just-in-time inside
            # the first projection pass (see below) so the queue order
            # matches consumption order and the PE starts ~15us in ---------
            wq_s = weights.tile([128, HC, HQ * D], BF16)
            wkv_s = weights.tile([128, HC, 2 * D], BF16)
            cosf_s = const.tile([128, S], BF16)
            sinf_s = const.tile([128, S], BF16)
            wo_s = weights.tile([128, HQ, H], BF16)

            def rope(dst, tsl):
                """In-place RoPE on dst[:, tsl] ([128, 512] bf16 slice).
                Runs on gpsimd (SBUF-only) to keep the vector queue free
                for the attention softmax chain."""
                scr = ropes.tile([128, 512], BF16, tag="scr")
                nc.sync.dma_start(out=scr[0:64, :], in_=dst[64:128, tsl])
                nc.sync.dma_start(out=scr[64:128, :], in_=dst[0:64, tsl])
                nc.gpsimd.tensor_mul(dst[:, tsl], dst[:, tsl], cosf_s[:, tsl])
                nc.gpsimd.tensor_mul(scr, scr, sinf_s[:, tsl])
                nc.gpsimd.tensor_add(dst[:, tsl], dst[:, tsl], scr)

            # Both batches' activations stay resident; phases are merged
            # as proj(b0), proj(b1), attn(b0), attn(b1), out(b0), out(b1)
            # so the second attention starts on drained engine queues.
            qTs, kTs, vTs, vnats, attnTs = [], [], [], [], []
            for b in range(B):
                qTs.append(acts.tile([128, HQ, S], BF16, tag=f"qT{b}",
                                     name=f"qT{b}"))
                kTs.append(acts.tile([128, S], BF16, tag=f"kT{b}",
                                     name=f"kT{b}"))
                vTs.append(acts.tile([128, S], BF16, tag=f"vT{b}",
                                     name=f"vT{b}"))
                vnats.append(acts.tile([128, KC, D + 1], BF16,
                                       tag=f"vnat{b}", name=f"vnat{b}"))
                attnTs.append(acts.tile([128, HQ, S], BF16, tag=f"attnT{b}",
                                        name=f"attnT{b}"))

            # ---- projections: qT/kT/vT = w.T @ x --------------------------
            for b in range(B):
                tok0 = b * S
                qT, kT, vT, vnat = qTs[b], kTs[b], vTs[b], vnats[b]
                nc.gpsimd.memset(vnat, 1.0)  # ones column for the denom

                for t in range(2):  # two 512-token chunks per batch
                    tsl = slice(t * 512, (t + 1) * 512)
                    psq = [psmain.tile([128, 512], F32, tag="ps",
                                       name=f"psq{_d}") for _d in range(HQ)]
                    psk = psmain.tile([128, 512], F32, tag="ps")
                    psv = psmain.tile([128, 512], F32, tag="ps")
                    for hcp in range(HC // 2):  # two 128-chunks per DMA
                        if b == 0 and t == 0:
                            # weights just-in-time, in consumption order
                            nc.sync.dma_start(
                                out=wq_s[:, 2 * hcp:2 * hcp + 2, :],
                                in_=wq_r[:, 2 * hcp:2 * hcp + 2, :])
                            nc.sync.dma_start(
                                out=wkv_s[:, 2 * hcp:2 * hcp + 2, :],
                                in_=wkv_r[:, 2 * hcp:2 * hcp + 2, :])
                        xp = stream.tile([128, 2, 512], BF16, tag="xp")
                        nc.sync.dma_start(
                            out=xp,
                            in_=xT_r[:, 2 * hcp:2 * hcp + 2,
                                     tok0 + t * 512: tok0 + (t + 1) * 512],
                        )
                        for sub in range(2):
                            hc = 2 * hcp + sub
                            first, last = hc == 0, hc == HC - 1
                            for d in range(HQ):
                                nc.tensor.matmul(
                                    psq[d],
                                    wq_s[:, hc, d * 128:(d + 1) * 128],
                                    xp[:, sub, :],
                                    start=first, stop=last,
                                )
                            nc.tensor.matmul(psk, wkv_s[:, hc, 0:128],
                                             xp[:, sub, :],
                                             start=first, stop=last)
                            nc.tensor.matmul(psv, wkv_s[:, hc, 128:256],
                                             xp[:, sub, :],
                                             start=first, stop=last)
                    if b == 0 and t == 0:
                        nc.sync.dma_start(out=cosf_s, in_=cosf)
                        nc.sync.dma_start(out=sinf_s, in_=sinf)
                    # evacuate + rope; k first so attention unblocks
                    # earliest. Evacs split scalar/vector so neither queue
                    # backlogs ahead of the attention softmax chain.
                    nc.vector.tensor_copy(kT[:, tsl], psk)
                    rope(kT, tsl)
                    nc.vector.tensor_copy(vT[:, tsl], psv)
                    # v natural [tok, d | 1] via PE transpose, per t-chunk
                    tp = psmain.tile([128, 512], BF16, tag="tpb", bufs=1)
                    for i in range(4):
                        kc = t * 4 + i
                        nc.tensor.transpose(
                            tp[:, i * 128:(i + 1) * 128],
                            vT[:, kc * 128:(kc + 1) * 128],
                            ident,
                        )
                    nc.vector.tensor_copy(vnat[:, t * 4:(t + 1) * 4, 0:128],
                                          tp)
                    for d in range(HQ):
                        if d < 2:
                            nc.scalar.copy(qT[:, d, tsl], psq[d])
                        else:
                            nc.vector.tensor_copy(qT[:, d, tsl], psq[d])
                        rope(qT[:, d, :], tsl)

                if b == 0:
                    # wo lands well before the first out-projection
                    for i in range(8):
                        nc.sync.dma_start(
                            out=wo_s[:, :, i * 512:(i + 1) * 512],
                            in_=wo_r[:, :, i * 512:(i + 1) * 512])

            # ---- attention: scores transposed, PV, renorm -----------------
            def scores(b, hh, qc):
                """sT[k,q] per k-block -> exp -> probsT (unnormalized)."""
                pt = probs.tile([128, KC, 512], BF16, tag="pt")
                q0 = qc * 512
                for kc in range(qc * 4 + 4):
                    qoff = max(0, kc * 128 - q0)  # causal column start
                    st = psmain.tile([128, 512], F32, tag="ps")
                    nc.tensor.matmul(
                        st[:, qoff:512],
                        kTs[b][:, kc * 128:(kc + 1) * 128],
                        qTs[b][:, hh, q0 + qoff:q0 + 512],
                        start=True, stop=True,
                    )
                    if kc * 128 >= q0:  # diagonal block: causal mask
                        nc.vector.tensor_add(
                            st[:, qoff:qoff + 128],
                            st[:, qoff:qoff + 128],
                            maskT,
                        )
                    nc.scalar.activation(
                        pt[:, kc, qoff:512],
                        st[:, qoff:512],
                        mybir.ActivationFunctionType.Exp,
                        scale=SCALE,
                    )
                return pt

            def pv_tail(b, hh, qc, pt):
                """PV per q-block, renorm by the free denominator,
                transpose back into attnT layout."""
                q0 = qc * 512
                ans = []
                for qr in range(4):
                    qb = qc * 4 + qr
                    pa = psmain.tile([128, 512], F32, tag="ps")
                    for kc in range(qb + 1):
                        nc.tensor.matmul(
                            pa[:, 0:D + 1],
                            pt[:, kc, qr * 128:(qr + 1) * 128],
                            vnats[b][:, kc, :],
                            start=(kc == 0), stop=(kc == qb),
                        )
                    rec = stats.tile([128, 1], F32, tag="st")
                    nc.vector.reciprocal(rec, pa[:, D:D + 1])
                    an = anpool.tile([128, 128], BF16, tag="an")
                    nc.vector.tensor_scalar_mul(an, pa[:, 0:D], rec)
                    ans.append(an)
                tp = psmain.tile([128, 512], BF16, tag="tpb", bufs=1)
                for qr in range(4):
                    nc.tensor.transpose(
                        tp[:, qr * 128:(qr + 1) * 128], ans[qr], ident)
                nc.vector.tensor_copy(attnTs[b][:, hh, q0:q0 + 512], tp)

            prev = None
            for b in range(B):
                for qc in range(2):
                    for hh in range(HQ):
                        pt = scores(b, hh, qc)
                        if prev is not None:
                            pv_tail(*prev)
                        prev = (b, hh, qc, pt)
            pv_tail(*prev)

            # ---- output projection: out[tok, :] += attnT.T @ wo -----------
            for b in range(B):
                tok0 = b * S
                attnT = attnTs[b]
                for tb in range(QB):
                    ev = evpool.tile([128, H], BF16, tag="ev")
                    for ncol in range(8):
                        po = psmain.tile([128, 512], F32, tag="ps")
                        for d in range(HQ):
                            nc.tensor.matmul(
                                po,
                                attnT[:, d, tb * 128:(tb + 1) * 128],
                                wo_s[:, d, ncol * 512:(ncol + 1) * 512],
                                start=(d == 0), stop=(d == HQ - 1),
                            )
                        if ncol % 2 == 0:
                            nc.scalar.copy(ev[:, ncol * 512:(ncol + 1) * 512],
                                           po)
                        else:
                            nc.vector.tensor_copy(
                                ev[:, ncol * 512:(ncol + 1) * 512], po)
                    # split across DMA queues; last tile finer for the tail
                    nsp = 8 if (b == B - 1 and tb == QB - 1) else 4
                    w = H // nsp
                    for i in range(nsp):
                        nc.sync.dma_start(
                            out=out[tok0 + tb * 128: tok0 + (tb + 1) * 128,
                                    i * w:(i + 1) * w],
                            in_=ev[:, i * w:(i + 1) * w],
                        )

    nc.compile()
    return nc


_NC = None


def _get_nc():
    global _NC
    if _NC is None:
        _NC = build_program()
    return _NC


def make_in_maps(x, wq, wk, wv, wo, freqs_cos, freqs_sin):
    bf = ml_dtypes.bfloat16
    x = np.asarray(x, np.float32)
    xT = np.ascontiguousarray(x.reshape(NT, H).T.astype(bf))
    cosT = np.asarray(freqs_cos, np.float32).T
    sinT = np.asarray(freqs_sin, np.float32).T
    cosf = np.ascontiguousarray(np.concatenate([cosT, cosT], 0).astype(bf))
    sinf = np.ascontiguousarray(np.concatenate([-sinT, sinT], 0).astype(bf))
    wq = np.asarray(wq, np.float32).astype(bf)
    wk = np.asarray(wk, np.float32).astype(bf)
    wv = np.asarray(wv, np.float32).astype(bf)
    wo = np.asarray(wo, np.float32).astype(bf)
    in_maps = []
    for c in range(8):
        in_maps.append({
            "xT": xT,
            "wq": np.ascontiguousarray(wq[:, c * 512:(c + 1) * 512]),
            "wkv": np.ascontiguousarray(
                np.concatenate([wk[:, c * 128:(c + 1) * 128],
                                wv[:, c * 128:(c + 1) * 128]], axis=1)),
            "wo": np.ascontiguousarray(wo[c * 512:(c + 1) * 512, :]),
            "cosf": cosf,
            "sinf": sinf,
        })
    return in_maps


def kernel(x, wq, wk, wv, wo, freqs_cos, freqs_sin, start_pos=0, **_):
    nc = _get_nc()
    in_maps = make_in_maps(x, wq, wk, wv, wo, freqs_cos, freqs_sin)
    res = run_bass_kernel_spmd(nc, in_maps, list(range(8)))
    acc = res.results[0]["out"].astype(np.float32)
    for c in range(1, 8):
        acc = acc + res.results[c]["out"].astype(np.float32)
    return acc.reshape(B, S, H)
